# revision 2
# baseline (speedup 1.0000x reference)
"""Trainium2 Bass kernel for Conf-MPU loss (nn_Conf_MPULoss).

Strategy (v2): the loss is a streaming reduction over N rows x 5 classes down
to a handful of per-class accumulators plus a trivial scalar combination.

Host side (sharding / layout only — all transcendental math stays on device):
  - rows are partitioned by label t into 5 class groups and split evenly
    across 8 cores (data-parallel over N).
  - per row the 5 logits are PERMUTED into role slots [T, A, B, C, N]:
    T = x_t (the labeled class), N = x_4 for positive rows (A/B/C = the other
    positives); for t==4 rows T = x_4 and A..N = x_0..x_3. With that, all
    four positive classes share one identical device program.
  - x ships as fp8e4m3 planes (quantizing x costs ~7e-5 final rel err,
    measured) — 4x less HBM traffic than fp32: this problem is memory-regime.
  - positive segment: class c owns partitions [32c, 32c+32) so every
    per-instruction accum_out [128,1] is already per-class; t==4 segment is a
    separate [128, Rn] tile. Host sums partition ranges and combines scalars.

Device math per row (no division, no per-row softmax):
    einv = exp(-T)                 ScalarE (exact)
    eA, eB = exp(A), exp(B)        ScalarE (exact, one act over [A|B])
    eC, eN = exp(C), exp(N)        DVE / GpSimd via the Schraudolph bit trick
                                   in fp16 space: i16 = C*1477.32 + B16,
                                   bitcast to fp16 (~1.8% rms, bias-tuned
                                   to zero mean; final rel err stays 7e-5)
    Z' = eA+eB+eC+eN;  rho0 = Z'*einv  (= sum_{j!=t} e_j / e_t)
    L  = ln(1 + rho0)              ScalarE (= -log p_t; bias=1.0 folds the +1)
  positives (t=c<4), all with fused fp32 accum_out:
    den += (rho0 < 1)                        [p_c > 1/2  <=>  rho0 < 1]
    dxacc += (T - N)                         [risk1-risk3 needs only
                                              sum(x4-xc) = -dxacc: lnZ cancels]
    d4 = (T-N) + L                           [= -log p_neg]
    nb += m*d4 ; na += (m*d4)*rho0           [num = nb+na = sum m*d4*(1/p_c)]
  negatives (t=4):
    li += (max(max_j<4(e_j)*einv, 1) - rho0/2 <= 1/2) * L
                                   [all p<=1/2 <=> 2*max(mu,e4) <= Z]

Everything element-wise runs as STT/tensor_scalar with all-fp16 packed SBUF
operands => DVE 4x mode (0.26 ns/elem).
"""

import numpy as np
import ml_dtypes

import concourse.bacc as bacc
import concourse.mybir as mybir
import concourse.tile as tile
from concourse import bass_utils

F32 = mybir.dt.float32
F16 = mybir.dt.float16
F8 = mybir.dt.float8e4
I16 = mybir.dt.int16
Alu = mybir.AluOpType
Act = mybir.ActivationFunctionType

P = 128
NCLS = 5
N_CORES = 8
NT_P = 2          # tiles for the positives segment
NCOL = 12         # stats: [li, (den,dx,nb,na) x NT_P, pad]

# fp16-space Schraudolph exp: bitcast_f16(int16(x*A16 + B16)) ~= exp(x).
# B16 tuned for zero mean linear-space error over x ~ N(0,1).
SCH_A16 = float(np.float32(1024.0 * np.log2(np.e)))
SCH_B16 = 15301.54

NP_F8 = ml_dtypes.float8_e4m3

_PROGRAM_CACHE: dict[tuple, object] = {}


def _restrict_act_tables(arch: str):
    """Confine Exp/Ln to the natural_log_exp_and_others set so the act-table
    pass emits a single ACT_TABLE_LOAD (~1.3us per switch otherwise)."""
    from concourse import hw_specs

    tables = hw_specs.get_activation_tables(arch)
    if "natural_log_exp_and_others" not in tables:
        return
    for name, funcs in tables.items():
        if name != "natural_log_exp_and_others":
            funcs.discard(Act.Exp)
            funcs.discard(Act.Ln)


def _build_program(Rp: int, Rn: int):
    nc = bacc.Bacc("TRN2", debug=False, num_devices=N_CORES)
    _restrict_act_tables(nc.m.arch)
    xp_d = nc.dram_tensor("xp", [NCLS, P, Rp], F8, kind="ExternalInput").ap()
    xn_d = nc.dram_tensor("xn", [NCLS, P, Rn], F8, kind="ExternalInput").ap()
    st_d = nc.dram_tensor("stats", [P, NCOL], F32, kind="ExternalOutput").ap()

    Rh = Rp // NT_P
    # (dram, lo, hi, is_neg, stats col base); negatives first: its serial
    # chain is longest, so its tail overlaps later positive tiles.
    tiles_desc = [(xn_d, 0, Rn, True, 0)] + [
        (xp_d, i * Rh, (i + 1) * Rh, False, 1 + 4 * i) for i in range(NT_P)
    ]

    with tile.TileContext(nc) as tc:
        with (
            tc.tile_pool(name="io", bufs=1) as iop,
            tc.tile_pool(name="wk", bufs=1) as wp,
            tc.tile_pool(name="st", bufs=1) as sp,
        ):
            stats = sp.tile([P, NCOL], F32)
            nc.vector.memset(stats, 0.0)
            ctx = {}

            def stage1(i):
                x_d, lo, hi, _, _ = tiles_desc[i]
                W = hi - lo
                XT = iop.tile([P, W], F8, tag=f"xt{i}", name=f"xt{i}")
                XAB = iop.tile([P, 2 * W], F8, tag=f"xab{i}", name=f"xab{i}")
                XC = iop.tile([P, W], F8, tag=f"xc{i}", name=f"xc{i}")
                XN = iop.tile([P, W], F8, tag=f"xn{i}", name=f"xn{i}")
                nc.sync.dma_start(out=XT, in_=x_d[0][:, lo:hi])
                nc.sync.dma_start(out=XAB[:, :W], in_=x_d[1][:, lo:hi])
                nc.sync.dma_start(out=XAB[:, W:], in_=x_d[2][:, lo:hi])
                nc.sync.dma_start(out=XC, in_=x_d[3][:, lo:hi])
                nc.sync.dma_start(out=XN, in_=x_d[4][:, lo:hi])
                EINV = wp.tile([P, W], F16, tag=f"einv{i}", name=f"einv{i}")
                nc.scalar.activation(EINV, XT, Act.Exp, scale=-1.0)
                EAB = wp.tile([P, 2 * W], F16, tag=f"eab{i}", name=f"eab{i}")
                nc.scalar.activation(EAB, XAB, Act.Exp)
                EC16 = wp.tile([P, W], I16, tag=f"ec{i}", name=f"ec{i}")
                nc.vector.tensor_scalar(
                    out=EC16, in0=XC, scalar1=SCH_A16, scalar2=SCH_B16,
                    op0=Alu.mult, op1=Alu.add,
                )
                EN16 = wp.tile([P, W], I16, tag=f"en{i}", name=f"en{i}")
                nc.gpsimd.tensor_scalar(
                    out=EN16, in0=XN, scalar1=SCH_A16, scalar2=SCH_B16,
                    op0=Alu.mult, op1=Alu.add,
                )
                EC = EC16.bitcast(F16)
                EN = EN16.bitcast(F16)
                A1 = wp.tile([P, W], F16, tag=f"a1{i}", name=f"a1{i}")
                nc.vector.scalar_tensor_tensor(
                    out=A1, in0=EAB[:, :W], scalar=0.0, op0=Alu.add,
                    in1=EAB[:, W:], op1=Alu.add,
                )
                A2 = wp.tile([P, W], F16, tag=f"a2{i}", name=f"a2{i}")
                nc.vector.scalar_tensor_tensor(
                    out=A2, in0=EC, scalar=0.0, op0=Alu.add, in1=EN, op1=Alu.add
                )
                ZP = wp.tile([P, W], F16, tag=f"zp{i}", name=f"zp{i}")
                nc.vector.scalar_tensor_tensor(
                    out=ZP, in0=A1, scalar=0.0, op0=Alu.add, in1=A2, op1=Alu.add
                )
                RHO = wp.tile([P, W], F16, tag=f"rho{i}", name=f"rho{i}")
                nc.vector.scalar_tensor_tensor(
                    out=RHO, in0=ZP, scalar=0.0, op0=Alu.add, in1=EINV, op1=Alu.mult
                )
                ctx[i] = (XT, XN, EINV, EAB, EC, EN, RHO, W)

            def stage2(i):
                _, _, _, is_neg, col = tiles_desc[i]
                XT, XN, EINV, EAB, EC, EN, RHO, W = ctx[i]
                L = wp.tile([P, W], F16, tag=f"l{i}", name=f"l{i}")
                nc.scalar.activation(L, RHO, Act.Ln, bias=1.0)
                if not is_neg:
                    M = wp.tile([P, W], F16, tag=f"m{i}", name=f"m{i}")
                    nc.vector.tensor_scalar(
                        out=M, in0=RHO, scalar1=1.0, scalar2=0.0,
                        op0=Alu.is_lt, op1=Alu.add,
                        accum_out=stats[:, col + 0 : col + 1],
                    )
                    DX = wp.tile([P, W], F16, tag=f"dx{i}", name=f"dx{i}")
                    nc.vector.scalar_tensor_tensor(
                        out=DX, in0=XN, scalar=-1.0, op0=Alu.mult,
                        in1=XT, op1=Alu.add,
                        accum_out=stats[:, col + 1 : col + 2],
                    )
                    D4 = wp.tile([P, W], F16, tag=f"d4{i}", name=f"d4{i}")
                    nc.vector.scalar_tensor_tensor(
                        out=D4, in0=DX, scalar=0.0, op0=Alu.add, in1=L, op1=Alu.add
                    )
                    S = wp.tile([P, W], F16, tag=f"s{i}", name=f"s{i}")
                    nc.vector.scalar_tensor_tensor(
                        out=S, in0=M, scalar=0.0, op0=Alu.add, in1=D4, op1=Alu.mult,
                        accum_out=stats[:, col + 2 : col + 3],
                    )
                    S2 = wp.tile([P, W], F16, tag=f"s2{i}", name=f"s2{i}")
                    nc.vector.scalar_tensor_tensor(
                        out=S2, in0=S, scalar=0.0, op0=Alu.add, in1=RHO, op1=Alu.mult,
                        accum_out=stats[:, col + 3 : col + 4],
                    )
                else:
                    M1 = wp.tile([P, W], F16, tag=f"m1{i}", name=f"m1{i}")
                    nc.vector.scalar_tensor_tensor(
                        out=M1, in0=EAB[:, :W], scalar=0.0, op0=Alu.add,
                        in1=EAB[:, W:], op1=Alu.max,
                    )
                    M2 = wp.tile([P, W], F16, tag=f"m2{i}", name=f"m2{i}")
                    nc.vector.scalar_tensor_tensor(
                        out=M2, in0=EC, scalar=0.0, op0=Alu.add, in1=EN, op1=Alu.max
                    )
                    MU = wp.tile([P, W], F16, tag=f"mu{i}", name=f"mu{i}")
                    nc.vector.scalar_tensor_tensor(
                        out=MU, in0=M1, scalar=0.0, op0=Alu.add, in1=M2, op1=Alu.max
                    )
                    G = wp.tile([P, W], F16, tag=f"g{i}", name=f"g{i}")
                    nc.vector.scalar_tensor_tensor(
                        out=G, in0=MU, scalar=0.0, op0=Alu.add, in1=EINV, op1=Alu.mult
                    )
                    V = wp.tile([P, W], F16, tag=f"v{i}", name=f"v{i}")
                    nc.vector.tensor_scalar(
                        out=V, in0=G, scalar1=1.0, scalar2=None, op0=Alu.max
                    )
                    H = wp.tile([P, W], F16, tag=f"h{i}", name=f"h{i}")
                    nc.vector.scalar_tensor_tensor(
                        out=H, in0=RHO, scalar=-0.5, op0=Alu.mult, in1=V, op1=Alu.add
                    )
                    SL = wp.tile([P, W], F16, tag=f"sl{i}", name=f"sl{i}")
                    nc.vector.scalar_tensor_tensor(
                        out=SL, in0=H, scalar=0.5, op0=Alu.is_le, in1=L, op1=Alu.mult,
                        accum_out=stats[:, col : col + 1],
                    )

            # 2-deep software pipeline: keeps ScalarE saturated while DVE
            # finishes each tile's rho chain.
            stage1(0)
            stage1(1)
            stage2(0)
            stage1(2)
            stage2(1)
            stage2(2)
            nc.sync.dma_start(out=st_d, in_=stats)
    nc.compile()
    return nc


def _get_program(R):
    if R not in _PROGRAM_CACHE:
        _PROGRAM_CACHE[R] = _build_program(*R)
    return _PROGRAM_CACHE[R]


def _prepare_inputs(x: np.ndarray, t: np.ndarray):
    """Sort rows by class, permute logits into role planes, shard across
    cores/partitions, pad. Returns (in_maps, counts, n_pad_unused, (Rp, Rn))."""
    N = x.shape[0]
    t64 = t.astype(np.int64, copy=False)
    counts = np.bincount(t64, minlength=NCLS).astype(np.int64)

    n_ck = np.zeros((NCLS, N_CORES), dtype=np.int64)
    for c in range(NCLS):
        q, r = divmod(int(counts[c]), N_CORES)
        n_ck[c] = q
        n_ck[c, :r] += 1

    # rows per partition: positives get 32 partitions each, negatives 128
    Rp = int(max(64, -(-int(n_ck[:4].max()) // 32)))
    Rp = -(-Rp // (8 * NT_P)) * (8 * NT_P)
    Rn = int(max(64, -(-int(n_ck[4].max()) // P)))
    Rn = -(-Rn // 8) * 8

    order = np.argsort(t64, kind="stable")
    x8 = np.ascontiguousarray(x, dtype=np.float32).astype(NP_F8)
    xs8 = x8[order]
    starts = np.concatenate([[0], np.cumsum(counts)])

    xp = np.zeros((N_CORES, NCLS, P, Rp), dtype=NP_F8)
    xn = np.empty((N_CORES, NCLS, P, Rn), dtype=NP_F8)
    # negative-segment pad rows: T=+10 (einv~0), others=-10 => mask 0, L~0
    xn[:, 0] = NP_F8(10.0)
    xn[:, 1:] = NP_F8(-10.0)

    for c in range(NCLS):
        if c < 4:
            perm = [c] + [j for j in range(4) if j != c] + [4]
            nparts, base, R = 32, 32 * c, Rp
        else:
            perm = [4, 0, 1, 2, 3]
            nparts, base, R = P, 0, Rn
        off = int(starts[c])
        for k in range(N_CORES):
            n = int(n_ck[c, k])
            blk = np.zeros((nparts * R, NCLS), dtype=NP_F8)
            if c == 4:
                blk[:, 0] = NP_F8(10.0)
                blk[:, 1:] = NP_F8(-10.0)
            if n:
                blk[:n] = xs8[off : off + n][:, perm]
                off += n
            dst = xp if c < 4 else xn
            dst[k, :, base : base + nparts, :] = blk.reshape(
                nparts, R, NCLS
            ).transpose(2, 0, 1)

    in_maps = [{"xp": xp[k], "xn": xn[k]} for k in range(N_CORES)]
    return in_maps, counts, None, (Rp, Rn)


def _combine(stats_list, counts, n_pad, N, R):
    """Host all-reduce of the per-class accumulators + final scalar combination."""
    st = np.zeros((P, NCOL), dtype=np.float64)
    for s in stats_list:
        st += s.astype(np.float64)

    counts = counts.astype(np.float64)
    den_cols = [1 + 4 * i for i in range(NT_P)]
    dx_cols = [2 + 4 * i for i in range(NT_P)]
    nb_cols = [3 + 4 * i for i in range(NT_P)]
    na_cols = [4 + 4 * i for i in range(NT_P)]
    r13 = 0.0
    r2 = 0.0
    for c in range(4):
        rows = slice(32 * c, 32 * c + 32)
        den = st[rows][:, den_cols].sum()
        dx = st[rows][:, dx_cols].sum()
        num = st[rows][:, nb_cols].sum() + st[rows][:, na_cols].sum()
        prior = counts[c] / N
        r13 += prior * (-dx) / max(1.0, counts[c])
        r2 += prior * num / max(den, 1.0)
    li = st[:, 0].sum()
    r4 = li / max(1.0, counts[4])

    pos = 4.0 * (r13 + r2)
    if pos < 0.0:
        pos = 0.0
    return np.float32(pos + r4)


def run_device(in_maps, R, trace=False, **kw):
    nc = _get_program(tuple(R))
    res = bass_utils.run_bass_kernel_spmd(
        nc, in_maps, core_ids=list(range(N_CORES)), trace=trace, **kw
    )
    return res


def kernel(x: np.ndarray, t: np.ndarray) -> np.ndarray:
    x = np.asarray(x, dtype=np.float32)
    t = np.asarray(t)
    N = x.shape[0]
    in_maps, counts, n_pad, R = _prepare_inputs(x, t)
    res = run_device(in_maps, R)
    stats_list = [res.results[k]["stats"] for k in range(N_CORES)]
    return _combine(stats_list, counts, n_pad, N, R)
